# revision 16
# baseline (speedup 1.0000x reference)
"""nn_ApproximateEuclideanAttention — 8-core Trainium2 Bass kernel.

Sharding (per spec hint): data-parallel over batch (2) x tensor-parallel over
heads (16 -> 4 groups of 4), one shard per NeuronCore. Each core computes its
head-group's attention and the partial output projection H @ Wo[:,sl].T; the
host sums the 4 partials per batch (+bo) and casts to fp32.

Device dataflow (validated numerically in numpy first; mean rel err 5.8e-3
vs fp32 reference with bf16 rounding everywhere, budget 2e-2):
  - activations "T-land" (embed on partitions): K^T/Q^T = W^T.T @ x^T; V in
    N-land (seq on partitions) for the Z-reduction.
  - The per-row ||Q||^2 and all ||L||^2 terms cancel in out/norm, so
    Phi_Q' = exp(2 L Q^T / tau), Phi_K' = exp((2 K L^T - ||K||^2)/tau), and
    the 64x64 solve matrix is Wexp = exp(2 L L^T / tau):
      out = (Phi_Q' A Z') / (Phi_Q' A s'),  A = Wexp^{-1}
  - ||K||^2 folded into the E_K matmul as a second accumulating matmul with
    a const blockdiag(-0.5) moving operand; Phi via one Exp activation.
  - A via 20 fp32 Newton-Schulz iters (X0 = I/256), head-pairs packed into
    one 128-wide tile with tile_position'd 64x64 matmuls.
  - norm reciprocal via the fast custom-DVE approx; 1/norm broadcast to the
    64 rows of each head by a partition-step-0 SBUF->SBUF DMA, division
    fused into the H^T PSUM->SBUF eviction (tensor_mul).
"""

import numpy as np
import ml_dtypes

EMBED_DIM = 1024
NUM_HEADS = 16
HEAD_DIM = 64
NL = 64          # landmarks
N_CORES = 8
GROUPS = 4       # head groups -> 4 heads / 256 cols per core
C = 256          # local channel cols per core
N = 8192         # sequence length per batch
SLAB = 512
NSLAB = N // SLAB          # 16
NCH = SLAB // 128          # chunks per slab = 4
NS_ITERS = 20
DEBUG_DUMP = False

BF16 = ml_dtypes.bfloat16

LAST_RESULTS = None  # BassKernelResults of the most recent device run


# ---------------------------------------------------------------------------
# device program
# ---------------------------------------------------------------------------

def _build_bass(tau: float):
    import concourse.bass as bass
    import concourse.tile as tile
    from concourse import bacc, mybir

    f32 = mybir.dt.float32
    bf = mybir.dt.bfloat16
    Exp = mybir.ActivationFunctionType.Exp
    Square = mybir.ActivationFunctionType.Square
    Copy = mybir.ActivationFunctionType.Copy

    nc = bacc.Bacc("TRN2", target_bir_lowering=False, debug=False,
                   num_devices=N_CORES)

    xT_d = nc.dram_tensor("xT", [EMBED_DIM, N], bf, kind="ExternalInput")
    xlT_d = nc.dram_tensor("xlT", [EMBED_DIM, NL], bf, kind="ExternalInput")
    wkT_d = nc.dram_tensor("wkT", [EMBED_DIM, C], bf, kind="ExternalInput")
    wqT_d = nc.dram_tensor("wqT", [EMBED_DIM, C], bf, kind="ExternalInput")
    wvT_d = nc.dram_tensor("wvT", [EMBED_DIM, C], bf, kind="ExternalInput")
    woT_d = nc.dram_tensor("woT", [C, EMBED_DIM], bf, kind="ExternalInput")
    sqsel_d = nc.dram_tensor("sqsel", [128, 128], bf, kind="ExternalInput")
    id2w_d = nc.dram_tensor("id2w", [128, 128], f32, kind="ExternalInput")
    ones_d = nc.dram_tensor("onesc", [128, 1], bf, kind="ExternalInput")
    out_d = nc.dram_tensor("opart", [N, EMBED_DIM], bf, kind="ExternalOutput")

    # dram views
    xT_r = xT_d.ap().rearrange("(ci p) n -> p ci n", p=128)       # (128,8,N)
    xlT_r = xlT_d.ap().rearrange("(ci p) l -> p ci l", p=128)     # (128,8,64)
    wk_r = wkT_d.ap().rearrange("(ci p) e -> p ci e", p=128)      # (128,8,256)
    wq_r = wqT_d.ap().rearrange("(ci p) e -> p ci e", p=128)
    wv_r = wvT_d.ap().rearrange("(ci p) e -> p ci e", p=128)
    wo_r = woT_d.ap().rearrange("(ct p) e -> p ct e", p=128)      # (128,2,1024)
    out_r = out_d.ap().rearrange("(s c p) e -> s p c e", p=128, c=NCH)

    sc = 2.0 / tau

    with tile.TileContext(nc) as tc:
        import contextlib
        ctx = contextlib.ExitStack()
        with ctx:
            singles = ctx.enter_context(tc.tile_pool(name="singles", bufs=1))
            big = ctx.enter_context(tc.tile_pool(name="big", bufs=1))
            slabs = ctx.enter_context(tc.tile_pool(name="slabs", bufs=2))
            nspool = ctx.enter_context(tc.tile_pool(name="nspool", bufs=2))
            ps_big = ctx.enter_context(
                tc.tile_pool(name="ps_big", bufs=2, space="PSUM"))
            ps_wide = ctx.enter_context(
                tc.tile_pool(name="ps_wide", bufs=2, space="PSUM"))
            ps_z = ctx.enter_context(
                tc.tile_pool(name="ps_z", bufs=2, space="PSUM"))

            # ---- stage 0: weights + consts -------------------------------
            wk_sb = singles.tile([128, 8, C], bf)
            wq_sb = singles.tile([128, 8, C], bf)
            wv_sb = singles.tile([128, 8, C], bf)
            wo_sb = singles.tile([128, 2, EMBED_DIM], bf)
            sqsel_sb = singles.tile([128, 128], bf)
            id2w_sb = singles.tile([128, 128], f32)
            ones_sb = singles.tile([128, 1], bf)
            xlT_sb = singles.tile([128, 8, NL], bf)
            nc.sync.dma_start(out=wk_sb[:], in_=wk_r)
            nc.sync.dma_start(out=wq_sb[:], in_=wq_r)
            nc.sync.dma_start(out=wv_sb[:], in_=wv_r)
            nc.sync.dma_start(out=wo_sb[:], in_=wo_r)
            nc.sync.dma_start(out=sqsel_sb[:], in_=sqsel_d.ap())
            nc.sync.dma_start(out=id2w_sb[:], in_=id2w_d.ap())
            nc.sync.dma_start(out=ones_sb[:], in_=ones_d.ap())
            nc.sync.dma_start(out=xlT_sb[:], in_=xlT_r)

            # warmup: absorb the const-bias-AP DMA wait into one tiny ACT op
            # (walrus allows only a couple of sync waits per instruction)
            warm = singles.tile([1, 1], f32)
            nc.scalar.activation(warm[:], id2w_sb[0:1, 0:1], Exp)

            # ---- stage 1: landmarks L^T (256, 64) ------------------------
            LT_sb = singles.tile([128, 2, NL], bf)
            for co in range(2):
                L_ps = ps_big.tile([128, NL], f32, tag="psb")
                for ci in range(8):
                    nc.tensor.matmul(
                        L_ps[:], wk_sb[:, ci, co * 128:(co + 1) * 128],
                        xlT_sb[:, ci, :], start=(ci == 0), stop=(ci == 7))
                nc.vector.tensor_copy(LT_sb[:, co, :], L_ps[:])

            # blockdiag(L^T) per pair (for S_Q lhsT and E_K rhs)
            bdl = singles.tile([128, 2, 128], bf)
            nc.vector.memset(bdl[:], 0.0)
            for t in range(2):
                nc.vector.tensor_copy(bdl[0:64, t, 0:64], LT_sb[0:64, t, :])
                nc.vector.tensor_copy(bdl[64:128, t, 64:128], LT_sb[64:128, t, :])

            # ---- stage 2: Wexp + Newton-Schulz inverse -------------------
            # Everything blockdiag per pair: blockdiag x blockdiag stays
            # blockdiag through plain full-128-contraction matmuls, so no
            # tile_position and no per-iter repacks are needed.
            W_ps = ps_big.tile([128, 128], f32, tag="psb")
            for t in range(2):
                nc.tensor.matmul(W_ps[:, 64 * t:64 * t + 64],
                                 bdl[:, t, :], LT_sb[:, t, :])
            Wf_sb = singles.tile([128, 128], f32)
            nc.scalar.activation(Wf_sb[:], W_ps[:], Exp, scale=sc)
            W_bd = [singles.tile([128, 128], f32, tag=f"wbd{t}", name=f"wbd{t}")
                    for t in range(2)]
            for t in range(2):
                nc.vector.memset(W_bd[t][:], 0.0)
                nc.vector.tensor_copy(W_bd[t][0:64, 0:64],
                                      Wf_sb[0:64, 64 * t:64 * t + 64])
                nc.vector.tensor_copy(W_bd[t][64:128, 64:128],
                                      Wf_sb[64:128, 64 * t:64 * t + 64])

            X_ping = [singles.tile([128, 128], f32, tag=f"xa{t}", name=f"xa{t}")
                      for t in range(2)]
            X_pong = [singles.tile([128, 128], f32, tag=f"xb{t}", name=f"xb{t}")
                      for t in range(2)]
            for t in range(2):
                nc.vector.tensor_scalar_mul(X_ping[t][:], id2w_sb[:],
                                            1.0 / 512.0)
            cur, nxt = X_ping, X_pong
            for it in range(NS_ITERS):
                for t in range(2):
                    P_ps = ps_big.tile([128, 128], f32, tag="psb")
                    nc.tensor.matmul(P_ps[:], W_bd[t][:], cur[t][:])
                    G_sb = nspool.tile([128, 128], f32, tag="nsg")
                    nc.vector.tensor_sub(G_sb[:], id2w_sb[:], P_ps[:])
                    Xp_ps = ps_big.tile([128, 128], f32, tag="psb")
                    nc.tensor.matmul(Xp_ps[:], cur[t][:], G_sb[:])
                    nc.vector.tensor_copy(nxt[t][:], Xp_ps[:])
                cur, nxt = nxt, cur
            M_bd = cur  # fp32 blockdiag inverse per pair

            # ---- stage 3: streaming projections + Phi + Z ----------------
            phiQ_sb = big.tile([128, 2, N], bf)
            Zacc_a = singles.tile([128, 258], f32)
            Zacc_b = singles.tile([128, 258], f32)
            for s in range(NSLAB):
                nsl = slice(s * SLAB, (s + 1) * SLAB)
                xts = slabs.tile([128, 8, SLAB], bf, tag="xts")
                nc.sync.dma_start(out=xts[:], in_=xT_r[:, :, nsl])

                # K^T and squares
                KT = slabs.tile([128, 2, SLAB], bf, tag="kt")
                sqKT = slabs.tile([128, 2, SLAB], bf, tag="sqkt")
                for co in range(2):
                    K_ps = ps_big.tile([128, SLAB], f32, tag="psb")
                    for ci in range(8):
                        nc.tensor.matmul(
                            K_ps[:], wk_sb[:, ci, co * 128:(co + 1) * 128],
                            xts[:, ci, :], start=(ci == 0), stop=(ci == 7))
                    nc.vector.tensor_copy(KT[:, co, :], K_ps[:])
                    nc.scalar.activation(sqKT[:, co, :], K_ps[:], Square)

                # Q^T -> Phi_Q'
                QT = slabs.tile([128, 2, SLAB], bf, tag="qt")
                for co in range(2):
                    Q_ps = ps_big.tile([128, SLAB], f32, tag="psb")
                    for ci in range(8):
                        nc.tensor.matmul(
                            Q_ps[:], wq_sb[:, ci, co * 128:(co + 1) * 128],
                            xts[:, ci, :], start=(ci == 0), stop=(ci == 7))
                    nc.vector.tensor_copy(QT[:, co, :], Q_ps[:])
                for t in range(2):
                    SQ_ps = ps_big.tile([128, SLAB], f32, tag="psb")
                    nc.tensor.matmul(SQ_ps[:], bdl[:, t, :], QT[:, t, :])
                    nc.scalar.activation(phiQ_sb[:, t, nsl], SQ_ps[:], Exp,
                                         scale=sc)

                # V (N-land)
                V_ps = ps_wide.tile([128, 4 * C], f32, tag="psw")
                for c in range(NCH):
                    for ci in range(8):
                        nc.tensor.matmul(
                            V_ps[:, c * C:(c + 1) * C],
                            xts[:, ci, c * 128:(c + 1) * 128],
                            wv_sb[:, ci, :], start=(ci == 0), stop=(ci == 7))
                Vb = slabs.tile([128, NCH, C], bf, tag="vb")
                nc.vector.tensor_copy(Vb[:], V_ps[:])

                # E_K (N-land) -> Phi_K'
                E_ps = ps_wide.tile([128, 4 * C], f32, tag="psw")
                for c in range(NCH):
                    for t in range(2):
                        cs = slice(c * C + 128 * t, c * C + 128 * t + 128)
                        nc.tensor.matmul(E_ps[:, cs],
                                         KT[:, t, c * 128:(c + 1) * 128],
                                         bdl[:, t, :], start=True, stop=False)
                        nc.tensor.matmul(E_ps[:, cs],
                                         sqKT[:, t, c * 128:(c + 1) * 128],
                                         sqsel_sb[:], start=False, stop=True)
                phiK = slabs.tile([128, NCH, C], bf, tag="phik")
                nc.scalar.activation(phiK[:], E_ps[:], Exp, scale=sc)

                # Z for this slab (short PSUM accumulation groups), then
                # accumulated across slabs in SBUF (ping-pong DVE adds)
                # NOTE: start=True clears has_written for the WHOLE PSUM
                # bank, so emit exactly one start per bank: later ranges'
                # first writes hit unmarked elements and overwrite; their
                # subsequent writes accumulate.
                Z_ps = ps_z.tile([128, 258], f32, tag="zslab")
                for c in range(NCH):
                    for t in range(2):
                        zc = 129 * t
                        nc.tensor.matmul(
                            Z_ps[:, zc:zc + 128],
                            phiK[:, c, 128 * t:128 * t + 128],
                            Vb[:, c, 128 * t:128 * t + 128],
                            start=(c == 0 and t == 0),
                            stop=(c == NCH - 1 and t == 1),
                            skip_group_check=True)
                        nc.tensor.matmul(
                            Z_ps[:, zc + 128:zc + 129],
                            phiK[:, c, 128 * t:128 * t + 128],
                            ones_sb[:],
                            start=False,
                            stop=False,
                            skip_group_check=True)
                if s == 0:
                    nc.vector.tensor_copy(Zacc_a[:], Z_ps[:])
                elif s % 2 == 1:
                    nc.vector.tensor_add(Zacc_b[:], Z_ps[:], Zacc_a[:])
                else:
                    nc.vector.tensor_add(Zacc_a[:], Z_ps[:], Zacc_b[:])

            # ---- stage 4: solve application + norm -----------------------
            Zs_sb = Zacc_b if NSLAB % 2 == 0 else Zacc_a
            # pack valid Z blocks so blockdiag(M) applies in one matmul/pair:
            # Zp cols [65t:65t+64] rows 0:64 <- Z0, rows 64:128 <- Z1; col
            # 65t+64 <- s (both halves valid).
            Zp_sb = singles.tile([128, 130], f32)
            for t in range(2):
                nc.vector.tensor_copy(Zp_sb[0:64, 65 * t:65 * t + 64],
                                      Zs_sb[0:64, 129 * t:129 * t + 64])
                nc.vector.tensor_copy(Zp_sb[64:128, 65 * t:65 * t + 64],
                                      Zs_sb[64:128, 129 * t + 64:129 * t + 128])
                nc.vector.tensor_copy(Zp_sb[:, 65 * t + 64:65 * t + 65],
                                      Zs_sb[:, 129 * t + 128:129 * t + 129])
            Y_ps = ps_big.tile([128, 130], f32, tag="psb")
            for t in range(2):
                nc.tensor.matmul(Y_ps[:, 65 * t:65 * t + 65],
                                 M_bd[t][:], Zp_sb[:, 65 * t:65 * t + 65])
            Yb_sb = singles.tile([128, 130], bf)
            nc.vector.tensor_copy(Yb_sb[:], Y_ps[:])
            # blockdiag(Y) per pair for the H matmuls
            Y_bd = singles.tile([128, 2, 128], bf)
            nc.vector.memset(Y_bd[:], 0.0)
            for t in range(2):
                nc.vector.tensor_copy(Y_bd[0:64, t, 0:64],
                                      Yb_sb[0:64, 65 * t:65 * t + 64])
                nc.vector.tensor_copy(Y_bd[64:128, t, 64:128],
                                      Yb_sb[64:128, 65 * t:65 * t + 64])

            selY = []
            for t in range(2):
                sl_t = singles.tile([128, 4], bf, tag=f"sely{t}", name=f"sely{t}")
                nc.vector.memset(sl_t[:], 0.0)
                for hh in range(2):
                    r = slice(64 * hh, 64 * hh + 64)
                    nc.vector.tensor_copy(
                        sl_t[r, 2 * t + hh:2 * t + hh + 1],
                        Yb_sb[r, 65 * t + 64:65 * t + 65])
                selY.append(sl_t)

            rnorm_f = big.tile([4, N], f32)
            for s in range(NSLAB):
                nsl = slice(s * SLAB, (s + 1) * SLAB)
                n_ps = ps_big.tile([4, SLAB], f32, tag="psb")
                for t in range(2):
                    nc.tensor.matmul(n_ps[:], selY[t][:],
                                     phiQ_sb[:, t, nsl],
                                     start=(t == 0), stop=(t == 1))
                nc.vector.reciprocal_approx_fast(out=rnorm_f[:, nsl],
                                                 in_=n_ps[:])
            if DEBUG_DUMP:
                zdbg = nc.dram_tensor("zdbg", [128, 258], f32,
                                      kind="ExternalOutput")
                ydbg = nc.dram_tensor("ydbg", [128, 130], f32,
                                      kind="ExternalOutput")
                mdbg = nc.dram_tensor("mdbg", [128, 256], f32,
                                      kind="ExternalOutput")
                rdbg = nc.dram_tensor("rdbg", [4, N], f32,
                                      kind="ExternalOutput")
                ybf = singles.tile([128, 130], f32)
                nc.vector.tensor_copy(ybf[:], Yb_sb[:])
                nc.sync.dma_start(out=zdbg.ap(), in_=Zs_sb[:])
                nc.sync.dma_start(out=ydbg.ap(), in_=ybf[:])
                for t in range(2):
                    nc.sync.dma_start(out=mdbg.ap()[:, 128*t:128*t+128],
                                      in_=M_bd[t][:])
                nc.sync.dma_start(out=rdbg.ap(), in_=rnorm_f[:])

            rnorm_b = big.tile([4, N], bf)
            nc.vector.tensor_copy(rnorm_b[:], rnorm_f[:])

            # broadcast 1/norm over each head's 64 rows: SBUF partition dims
            # can't have step 0, so bounce through DRAM (DRAM src APs can).
            rn_dram = nc.dram_tensor("rnbounce", [4, N], bf)
            nc.sync.dma_start(out=rn_dram.ap(), in_=rnorm_b[:])
            rnE = big.tile([128, 2, N], bf)
            rn_ap = rn_dram.ap()
            for t in range(2):
                for hh in range(2):
                    src = bass.AP(tensor=rn_ap.tensor,
                                  offset=(2 * t + hh) * N,
                                  ap=[[0, 64], [1, N]])
                    nc.gpsimd.dma_start(out=rnE[64 * hh:64 * hh + 64, t, :],
                                        in_=src)

            # ---- stage 5: H^T, divide, output projection -----------------
            for s in range(NSLAB):
                nsl = slice(s * SLAB, (s + 1) * SLAB)
                HT = slabs.tile([128, 2, SLAB], bf, tag="ht")
                for t in range(2):
                    H_ps = ps_big.tile([128, SLAB], f32, tag="psb")
                    nc.tensor.matmul(H_ps[:], Y_bd[:, t, :],
                                     phiQ_sb[:, t, nsl])
                    nc.vector.tensor_mul(HT[:, t, :], H_ps[:], rnE[:, t, nsl])
                O_ps = ps_wide.tile([128, EMBED_DIM], f32, tag="psw")
                oout = slabs.tile([128, NCH, EMBED_DIM], bf, tag="oout")
                for c in range(NCH):
                    for eh in range(2):
                        es = slice(eh * 512, eh * 512 + 512)
                        for ct in range(2):
                            nc.tensor.matmul(
                                O_ps[:, es],
                                HT[:, ct, c * 128:(c + 1) * 128],
                                wo_sb[:, ct, es],
                                start=(ct == 0), stop=(ct == 1))
                    if c % 2 == 0:
                        nc.vector.tensor_copy(oout[:, c, :], O_ps[:])
                    else:
                        nc.scalar.activation(oout[:, c, :], O_ps[:], Copy)
                nc.sync.dma_start(out=out_r[s], in_=oout[:])
    nc.compile()
    return nc


_NC_CACHE = None


def _get_nc(tau):
    global _NC_CACHE
    if _NC_CACHE is None:
        _NC_CACHE = _build_bass(tau)
    return _NC_CACHE


# ---------------------------------------------------------------------------
# host marshalling
# ---------------------------------------------------------------------------

def _consts():
    sqsel = np.zeros((128, 128), np.float32)
    sqsel[0:64, 0:64] = -0.5
    sqsel[64:128, 64:128] = -0.5
    id2w = 2.0 * np.eye(128, dtype=np.float32)
    onesc = np.ones((128, 1), np.float32)
    return (sqsel.astype(BF16), id2w, onesc.astype(BF16))


def _kernel_device(query, Wq, Wk, Wv, Wo, bo, tau, idx):
    global LAST_RESULTS
    from concourse.bass_utils import run_bass_kernel_spmd

    nc = _get_nc(tau)
    b, n, _ = query.shape

    sqsel, id2w, onesc = _consts()
    WkT = np.ascontiguousarray(Wk.T).astype(BF16)
    WqT = np.ascontiguousarray(Wq.T).astype(BF16)
    WvT = np.ascontiguousarray(Wv.T).astype(BF16)
    WoT = np.ascontiguousarray(Wo.T).astype(BF16)

    in_maps = []
    for bi in range(b):
        xT = np.ascontiguousarray(query[bi].T).astype(BF16)
        xlT = np.ascontiguousarray(query[bi][idx].T).astype(BF16)
        for g in range(GROUPS):
            sl = slice(g * C, (g + 1) * C)
            in_maps.append({
                "xT": xT,
                "xlT": xlT,
                "wkT": np.ascontiguousarray(WkT[:, sl]),
                "wqT": np.ascontiguousarray(WqT[:, sl]),
                "wvT": np.ascontiguousarray(WvT[:, sl]),
                "woT": np.ascontiguousarray(WoT[sl, :]),
                "sqsel": sqsel,
                "id2w": id2w,
                "onesc": onesc,
            })

    res = run_bass_kernel_spmd(nc, in_maps, core_ids=list(range(N_CORES)))
    LAST_RESULTS = res

    out = np.zeros((b, n, EMBED_DIM), np.float32)
    for bi in range(b):
        for g in range(GROUPS):
            out[bi] += res.results[bi * GROUPS + g]["opart"].astype(np.float32)
    out += bo
    return out


def _kernel_numpy(query, Wq, bq, Wk, bk, Wv, bv, Wo, bo, tau, idx):
    """Reference-faithful fallback (nonzero biases etc.)."""
    b, n, _ = query.shape
    out = np.zeros((b, n, EMBED_DIM), np.float32)
    for bi in range(b):
        x = query[bi]
        Q = (x @ Wq.T + bq).reshape(n, NUM_HEADS, HEAD_DIM).transpose(1, 0, 2)
        K = (x @ Wk.T + bk).reshape(n, NUM_HEADS, HEAD_DIM).transpose(1, 0, 2)
        V = (x @ Wv.T + bv).reshape(n, NUM_HEADS, HEAD_DIM).transpose(1, 0, 2)
        L = K[:, idx, :]
        def sqd(X, Lm):
            Xn = np.sum(X * X, -1, keepdims=True)
            Ln = np.sum(Lm * Lm, -1, keepdims=True)
            return np.maximum(Xn + np.swapaxes(Ln, -2, -1)
                              - 2.0 * np.einsum("hnd,hkd->hnk", X, Lm), 0.0)
        PhiQ = np.exp(-sqd(Q, L) / tau)
        PhiK = np.exp(-sqd(K, L) / tau)
        Wk_ = np.exp(-sqd(L, L) / tau) + 1e-6 * np.eye(NL, dtype=np.float32)
        Z = np.einsum("hnk,hnd->hkd", PhiK, V)
        Y = np.linalg.solve(Wk_, Z)
        ou = np.einsum("hnk,hkd->hnd", PhiQ, Y)
        sY = np.linalg.solve(Wk_, PhiK.sum(1)[..., None])
        nrm = np.maximum(np.einsum("hnk,hko->hno", PhiQ, sY), 1e-10)
        H = (ou / nrm).transpose(1, 0, 2).reshape(n, EMBED_DIM)
        out[bi] = H @ Wo.T
    return out + bo


def kernel(query, Wq, bq, Wk, bk, Wv, bv, Wo, bo, temperature, landmark_idx):
    query = np.asarray(query, dtype=np.float32)
    Wq, Wk, Wv, Wo = (np.asarray(w, np.float32) for w in (Wq, Wk, Wv, Wo))
    bq, bk, bv, bo = (np.asarray(x, np.float32) for x in (bq, bk, bv, bo))
    tau = float(np.asarray(temperature))
    idx = np.asarray(landmark_idx).astype(np.int64)

    if (query.shape != (2, N, EMBED_DIM) or idx.shape != (NL,)
            or np.any(bq) or np.any(bk) or np.any(bv)):
        return _kernel_numpy(query, Wq, bq, Wk, bk, Wv, bv, Wo, bo, tau, idx)
    return _kernel_device(query, Wq, Wk, Wv, Wo, bo, tau, idx).astype(
        np.float32, copy=False)
